# revision 93
# baseline (speedup 1.0000x reference)
"""Causal multi-head self-attention (RoPE) Trainium2 Bass kernel.

Contract: kernel(**inputs) takes the FULL unsharded inputs
  x [B=2, S=2048, D=1024] f32, qkv_w [3072, 1024] f32,
  out_w [1024, 1024] f32, token_positions [2048] i32
and returns the FULL output [2, 2048, 1024] f32.

Sharding: B (2) x head-groups (4 heads each) -> 8 cores.
Core c: batch c//4, heads 4*(c%4) .. 4*(c%4)+3.
Each core computes a partial output projection over its 256 local
head-dims; the host sums the 4 partials per batch.

Device-side design (v3 - fp8 DoubleRow scores):
  - Scores run as fp8e4m3 DoubleRow matmuls (0.5 PE cycles/out-col,
    2x bf16).  q/k live in a plane-major layout: head h occupies
    partitions 32h..32h+31; d-dim plane i = d//32 sits at free offset
    i*S.  The DR moving/stationary APs are [32, 2, n] with the plane
    pair in the free dim.  Everything else (exp/et, AV, normalize,
    out-projection) stays bf16 (psum f32); measured rel err 1.25e-2
    vs the 2e-2 gate.
  - HW constraint: consecutive DoubleRow matmuls at different
    tile_positions fault the PE; the stream therefore emits a bf16
    separator matmul (an AV half-flush, or a tiny dummy while the
    pipeline fills) between every DR score pair.
  - The q/k projection emits the plane layout directly: per plane i,
    one 128-col stationary covers all 4 heads' 32 plane-i dims, so a
    plane pair needs two matmuls per (t, j) - same total PE cost as
    v2.  Plane-0 sweeps all j at DMA pace; plane-1 for j=0 runs right
    after, and plane-1 for j=1..3 (+ its rope) weaves into the
    attention stream as background units.
  - RoPE per plane via DVE stream_shuffle (within-quadrant partition
    pair swap, no PE/psum hop): q' = q (.) cos_i + swap(q) (.) sin'_i
    with a sign-interleaved sin table; the add writes qp8/kp8 as fp8
    directly.  Tables are plane-specific, DMA'd in [128, 512] column
    chunks ordered so the j=0 chunks land first.
  - The v-projection (st 0..3 inline in the lead-in, the rest as
    background) and the out-projection fill PE slack in the
    Act(exp)-bound attention stream; the block order interleaves the
    PE-light (1,0) block into pass 0 to balance engines.
  - Scores are k-major (scores^T [sk, sq]); softmax skips the max
    subtraction; exp writes bf16 et tiles consumed by k-major AV
    ([65, 512] po accumulators, ones column = denominator); AVs trail
    the exp stream by PEND_DEPTH tiles; normalization via DVE
    reciprocal + gpsimd broadcast/mul into bf16 ao; causal masking
    multiplies one [128,256] triangular mask on diagonal et windows.
  - Tail: chunk-3 out-projection with psum tags rotated over every
    free bank (kc=0 halves run early), staged into 2-chunk tiles so
    only 4 output DMAs (625ns serial HWDGE desc-gen each) remain.
"""

import os
import sys

import numpy as np

_REPO_CANDIDATES = [
    "/opt/trn_rl_repo",
    "/root/.axon_site/_ro/trn_rl_repo",
]


def _ensure_repo_on_path():
    try:
        import concourse.bass  # noqa: F401
        return
    except ImportError:
        pass
    for p in _REPO_CANDIDATES:
        if os.path.isdir(p) and p not in sys.path:
            sys.path.insert(0, p)
    import concourse.bass  # noqa: F401


NUM_HEADS = 16
ROPE_THETA = 10000.0
D = 1024
DK = 64
H_LOC = 4          # heads per core
N_CORES = 8
N_WARM = 30        # PE warm-up dummy matmuls during DMA lead-in
PEND_DEPTH = 3     # AV staging depth behind the score/exp stream
ET_BUFS = 16       # et tile ring size
TAIL_DUMMIES = 8   # clock-keeper matmuls during the final normalize
POP_FRONT = 4      # pop background on every i below this index


# --------------------------------------------------------------------------
# Device program
# --------------------------------------------------------------------------

def build_nc(S=2048):
    """Build the per-core Bass program (SPMD, same on all 8 cores)."""
    _ensure_repo_on_path()
    import concourse.mybir as mybir
    from concourse import bacc
    from concourse.tile import TileContext
    from concourse.alu_op_type import AluOpType

    dt = mybir.dt
    f32, bf16, fp8 = dt.float32, dt.bfloat16, dt.float8e4
    DR = mybir.MatmulPerfMode.DoubleRow
    Exp = mybir.ActivationFunctionType.Exp
    MUL, ADD, DIV = AluOpType.mult, AluOpType.add, AluOpType.divide

    NC = S // 512    # 512-wide q-chunks
    NT = S // 128    # 128-wide s-tiles
    KD = D // 128    # d-chunks (contraction)

    nc = bacc.Bacc(None, target_bir_lowering=False, debug=False)

    xT = nc.dram_tensor("xT", [D, S], bf16, kind="ExternalInput")
    # plane-i q/k stationaries, SBUF layout [128, t*256]: per t-chunk
    # cols 0:128 = q plane-i (4 heads x 32), cols 128:256 = k plane-i
    wq0T = nc.dram_tensor("wq0T", [128, 256 * KD], bf16, kind="ExternalInput")
    wq1T = nc.dram_tensor("wq1T", [128, 256 * KD], bf16, kind="ExternalInput")
    wvT = nc.dram_tensor("wvT", [128, 256 * KD], bf16, kind="ExternalInput")
    woT = nc.dram_tensor("woT", [256, 1024], bf16, kind="ExternalInput")
    # plane trig tables: [sin0 S | cos0 S | sin1 S | cos1 S]
    cs2T = nc.dram_tensor("cs2T", [128, 4 * S], bf16, kind="ExternalInput")
    tri = nc.dram_tensor("tri", [128, 256], bf16, kind="ExternalInput")
    oT = nc.dram_tensor("oT", [D, S], bf16, kind="ExternalOutput")

    with TileContext(nc) as tc, \
         nc.allow_low_precision(reason="fp8 DR scores validated vs 2e-2 gate"):
      with tc.tile_pool(name="persist", bufs=1) as P:
        # ---- persistent SBUF tiles ----
        # plane-major fp8 q/k: head h at partitions 32h..32h+31,
        # plane i at free offset i*S
        qp8 = P.tile([128, 2 * S], fp8, name="qp8")
        kp8 = P.tile([128, 2 * S], fp8, name="kp8")
        vbig = P.tile([128, 260 * NT], bf16, name="vbig")
        wo_sb = [P.tile([128, 1024], bf16, name=f"wo{i}") for i in range(2)]
        ao = [P.tile([128, S], bf16, name=f"ao{p}") for p in range(2)]
        xt_sb = [P.tile([128, S], bf16, name=f"xt{t}") for t in range(KD)]
        wq_sb = [P.tile([128, 256 * KD], bf16, name=f"wqp{i}") for i in range(2)]
        wv_sb = P.tile([128, 256 * KD], bf16, name="wv_sb")
        cs_sb = P.tile([128, 4 * S], bf16, name="cs_sb")
        trit = P.tile([128, 256], bf16, name="trit")
        dmy = P.tile([128, 256], bf16, name="dmy")
        dmy_act = P.tile([1, 1], f32, name="dmy_act")

        def sinc(i, j):
            """sin plane-i col chunk j of cs_sb."""
            return cs_sb[:, (2 * i) * S + 512 * j:(2 * i) * S + 512 * (j + 1)]

        def cosc(i, j):
            return cs_sb[:, (2 * i + 1) * S + 512 * j:
                         (2 * i + 1) * S + 512 * (j + 1)]

        def cs_dma(i, j):
            off_s = (2 * i) * S + 512 * j
            off_c = (2 * i + 1) * S + 512 * j
            nc.sync.dma_start(out=cs_sb[:, off_s:off_s + 512],
                              in_=cs2T[:, off_s:off_s + 512])
            nc.sync.dma_start(out=cs_sb[:, off_c:off_c + 512],
                              in_=cs2T[:, off_c:off_c + 512])

        # ---- DMA issue order (SP queue, serial; critical path first) ----
        nc.sync.dma_start(out=wq_sb[0][:], in_=wq0T[:])
        nc.sync.dma_start(out=xt_sb[0][:, 0:1024], in_=xT[0:128, 0:1024])
        nc.sync.dma_start(out=xt_sb[0][:, 1024:S], in_=xT[0:128, 1024:S])
        for t in range(1, KD):
            nc.sync.dma_start(out=xt_sb[t][:], in_=xT[128 * t:128 * (t + 1), :])
        nc.sync.dma_start(out=wq_sb[1][:], in_=wq1T[:])
        # j=0 tables first (they gate attention start), then tri + the rest
        # in the order the background rope/score stream consumes them
        cs_dma(1, 0)
        cs_dma(0, 0)
        nc.sync.dma_start(out=trit[:], in_=tri[:])
        nc.sync.dma_start(out=wv_sb[:], in_=wvT[:])
        cs_dma(0, 1)
        cs_dma(1, 1)
        cs_dma(0, 2)
        cs_dma(1, 2)
        cs_dma(0, 3)
        cs_dma(1, 3)
        for i in range(2):
            nc.sync.dma_start(out=wo_sb[i][:],
                              in_=woT[128 * i:128 * (i + 1), :])
        with tc.tile_pool(name="rot", bufs=1) as RT:

            # ---- PE warm-up + Exp table preload during DMA lead-in ----
            # (dmy memset first: it heads the DVE queue and gates warm-up)
            nc.vector.memset(dmy[:, 0:128], 0.0)
            nc.scalar.activation(dmy_act[:], dmy[0:1, 0:1], Exp)
            # softmax-denominator ones columns of vbig (engine write: the
            # DMA path mis-tracks strided multi-dim writes)
            ones_cols = vbig[:].rearrange(
                "p (st h w) -> p st h w", st=NT, h=H_LOC)[:, :, :, 64:65]
            nc.vector.memset(ones_cols, 1.0)

            qs_tiles = {}
            # within-quadrant partition pair swap (RoPE rotate-half)
            SHUF = [m + 1 if m % 2 == 0 else m - 1 for m in range(32)]

            def qs_copy(pi, j, w, ps, eng):
                """psum -> sbuf copy; frees the projection bank early."""
                qs = RT.tile([128, 512], bf16, tag="qs", bufs=16,
                             name=f"qs_{pi}_{j}_{w}")
                if eng == "act":
                    nc.scalar.copy(qs[:], ps[:])
                else:
                    nc.vector.tensor_copy(qs[:], ps[:])
                qs_tiles[(pi, j, w)] = qs

            def rope_back(pi, j, w, t1_dve=False):
                """pair-swap shuffle + signed-sin/cos muls + fp8 add."""
                qs = qs_tiles.pop((pi, j, w))
                qsw = RT.tile([128, 512], bf16, tag=f"r0{w}", bufs=2,
                              name=f"qsw_{pi}_{j}_{w}")
                nc.vector.stream_shuffle(qsw[:], qs[:], SHUF)
                # cos-mul on gpsimd for bg units (keeps DVE short); the
                # inline j0 chains use DVE (gpsimd's 1.1us mul would sit on
                # the attention-start critical path)
                t1 = RT.tile([128, 512], bf16, tag=f"r1{w}", bufs=2,
                             name=f"rt1_{pi}_{j}_{w}")
                if t1_dve:
                    nc.vector.tensor_tensor(t1[:], qs[:], cosc(pi, j), MUL)
                else:
                    nc.gpsimd.tensor_tensor(t1[:], qs[:], cosc(pi, j), MUL)
                t2 = RT.tile([128, 512], bf16, tag=f"r2{w}", bufs=2,
                              name=f"rt2_{pi}_{j}_{w}")
                nc.vector.tensor_tensor(t2[:], qsw[:], sinc(pi, j), MUL)
                dst = qp8 if w == 0 else kp8
                nc.vector.tensor_tensor(
                    dst[:, S * pi + 512 * j:S * pi + 512 * (j + 1)],
                    t1[:], t2[:], ADD)

            def v_unit(st, pool, tagf):
                pv = pool.tile([128, 256], f32, tag=tagf, name=f"ps_v{st}")
                for t in range(KD):
                    nc.tensor.matmul(
                        pv[:], xt_sb[t][:, 128 * st:128 * (st + 1)],
                        wv_sb[:, 256 * t:256 * (t + 1)],
                        start=(t == 0), stop=(t == KD - 1))
                dstv = vbig[:, 260 * st:260 * (st + 1)].rearrange(
                    "p (h w) -> p h w", w=65)[:, :, 0:64]
                srcv = pv[:].rearrange("p (h w) -> p h w", w=64)
                # alternate copy engine: DVE is busy with rope/normalize
                if st % 2 == 0:
                    nc.vector.tensor_copy(dstv, srcv)
                else:
                    nc.scalar.copy(dstv, srcv)

            # ============ plane-0 sweep (all j) + plane-1 j=0 ============
            with tc.tile_pool(name="ps_proj", bufs=1, space="PSUM") as PSP:
                for w in range(N_WARM):
                    dp = PSP.tile([128, 128], f32, tag=f"p{'ab'[w % 2]}0",
                                  name=f"dmy_ps{w}")
                    nc.tensor.matmul(dp[:], dmy[:, 0:128], dmy[:, 0:128],
                                     start=True, stop=True)

                psQ = [PSP.tile([128, 512], f32, tag=f"pa{j}",
                                name=f"ps_q0_{j}") for j in range(NC)]
                psK = [PSP.tile([128, 512], f32, tag=f"pb{j}",
                                name=f"ps_k0_{j}") for j in range(NC)]
                for t in range(KD):
                    for j in range(NC):
                        sj = slice(512 * j, 512 * (j + 1))
                        nc.tensor.matmul(
                            psQ[j][:], wq_sb[0][:, 256 * t:256 * t + 128],
                            xt_sb[t][:, sj],
                            start=(t == 0), stop=(t == KD - 1))
                        nc.tensor.matmul(
                            psK[j][:], wq_sb[0][:, 256 * t + 128:256 * t + 256],
                            xt_sb[t][:, sj],
                            start=(t == 0), stop=(t == KD - 1))

                # j0 copies first (free pa0/pb0 for the plane-1 j0 mms and
                # keep the j0 rope chain at the head of the DVE/Act queues)
                qs_copy(0, 0, 0, psQ[0], "act")
                qs_copy(0, 0, 1, psK[0], "dve")

                # plane-1 j=0 (attention j0 needs both planes of chunk 0)
                ps1 = {}
                for w in range(2):
                    p1 = PSP.tile([128, 512], f32, tag=f"p{'ab'[w]}0",
                                  name=f"ps_{'qk'[w]}1_0")
                    for t in range(KD):
                        nc.tensor.matmul(
                            p1[:], wq_sb[1][:, 256 * t + 128 * w:
                                            256 * t + 128 * (w + 1)],
                            xt_sb[t][:, 0:512],
                            start=(t == 0), stop=(t == KD - 1))
                    ps1[w] = p1

                # rope j=0 inline (gates attention start): plane 0 then 1
                for w in range(2):
                    rope_back(0, 0, w, t1_dve=True)
                for w in range(2):
                    qs_copy(1, 0, w, ps1[w], "act" if w == 0 else "dve")
                for w in range(2):
                    rope_back(1, 0, w, t1_dve=True)

                # remaining plane-0 copies on Act (the DVE queue is busy
                # with the j0 rope chain that gates attention start)
                for j in range(1, NC):
                    qs_copy(0, j, 0, psQ[j], "act")
                    qs_copy(0, j, 1, psK[j], "act")

                # v units 0..3 fill the PE while DVE chews the j0 rope
                # chain (also keeps the PE clock ramped into attention)
                for st in range(4):
                    v_unit(st, PSP, f"p{'ab'[st % 2]}{st // 2 + 1}")

            # ==================== attention ===============================
            with tc.tile_pool(name="ps_att", bufs=1, space="PSUM") as PSA, \
                 tc.tile_pool(name="et_pool", bufs=1) as ET, \
                 tc.tile_pool(name="nrm_pool", bufs=1) as NP, \
                 tc.tile_pool(name="ostage", bufs=1) as OS:

                # ---- background units (fill spare PE slots) ----
                p1_state = {}

                def bg_v(st):
                    v_unit(st, PSA, "bg0" if st % 2 == 0 else "bg1")

                def bg_s2mm(j, w):
                    """plane-1 q|k projection for chunk j (bg in attention)."""
                    ps1 = PSA.tile([128, 512], f32, tag=f"bg{w}",
                                   name=f"ps_{'qk'[w]}1_{j}")
                    for t in range(KD):
                        sj = slice(512 * j, 512 * (j + 1))
                        nc.tensor.matmul(
                            ps1[:], wq_sb[1][:, 256 * t + 128 * w:
                                             256 * t + 128 * (w + 1)],
                            xt_sb[t][:, sj],
                            start=(t == 0), stop=(t == KD - 1))
                    p1_state[(j, w)] = ps1

                def bg_s2cp(j):
                    # both on DVE: Act is the exp bottleneck mid-attention
                    qs_copy(1, j, 0, p1_state.pop((j, 0)), "dve")
                    qs_copy(1, j, 1, p1_state.pop((j, 1)), "dve")

                def bg_rope(pi, j, w):
                    rope_back(pi, j, w)

                def o_unit(j, e, queue=None, copy_eng=None, tag=None,
                           tag_bufs=1):
                    sjj = slice(512 * j, 512 * (j + 1))
                    pf = PSA.tile([128, 512], f32,
                                  tag=tag or ("bg0" if e % 2 == 0 else "bg1"),
                                  bufs=tag_bufs if tag else 1,
                                  name=f"pf_{j}_{e}")
                    for kc in range(2):
                        nc.tensor.matmul(
                            pf[:], wo_sb[kc][:, 128 * e:128 * (e + 1)],
                            ao[kc][:, sjj],
                            start=(kc == 0), stop=(kc == 1))
                    ot = OS.tile([128, 512], bf16, tag="ot", bufs=4,
                                 name=f"ot_{j}_{e}")
                    if copy_eng == "act":
                        nc.scalar.copy(ot[:], pf[:])
                    else:
                        nc.vector.tensor_copy(ot[:], pf[:])
                    (queue or nc.sync).dma_start(
                        out=oT[128 * e:128 * (e + 1), sjj], in_=ot[:])

                # bg order interleaves plane-0/1 rope+proj for j=1..3 (block
                # j's scores read them; deadline = last pop of block j-1)
                # with v units (AV of tile st reads v st; deadline = its
                # flush).  Cadence: 2 pops/i in p0 j0+j1, 1/i in p0 j2+j3,
                # every other i in p1 (o units).
                background = [
                    (bg_rope, (0, 1, 0)), (bg_rope, (0, 1, 1)),
                    (bg_s2mm, (1, 0)), (bg_s2mm, (1, 1)), (bg_s2cp, (1,)),
                    (bg_rope, (1, 1, 0)), (bg_rope, (1, 1, 1)),
                    (bg_v, (4,)),
                    (bg_v, (5,)), (bg_v, (6,)), (bg_v, (7,)),
                    (bg_rope, (0, 2, 0)), (bg_rope, (0, 2, 1)),
                    (bg_s2mm, (2, 0)), (bg_s2mm, (2, 1)), (bg_s2cp, (2,)),
                    (bg_rope, (1, 2, 0)), (bg_rope, (1, 2, 1)),
                    (bg_v, (8,)), (bg_v, (9,)), (bg_v, (10,)), (bg_v, (11,)),
                    (bg_v, (12,)), (bg_v, (13,)), (bg_v, (14,)), (bg_v, (15,)),
                    (bg_rope, (0, 3, 0)), (bg_rope, (0, 3, 1)),
                    (bg_s2mm, (3, 0)), (bg_s2mm, (3, 1)), (bg_s2cp, (3,)),
                    (bg_rope, (1, 3, 0)), (bg_rope, (1, 3, 1)),
                ]

                def pop_bg():
                    if background:
                        fn, args = background.pop(0)
                        fn(*args)

                USE_DR = True

                def score_mm(p, hh, i, j, w0, ps):
                    """fp8 DoubleRow scores^T for head 2p+hh, k-tile i."""
                    h = 2 * p + hh
                    n_w = 512 - w0
                    lhsT = kp8[32 * h:32 * (h + 1), :].rearrange(
                        "p (i s) -> p i s", i=2)[:, :, 128 * i:128 * (i + 1)]
                    rhs = qp8[32 * h:32 * (h + 1), :].rearrange(
                        "p (i s) -> p i s", i=2)[:, :, 512 * j + w0:
                                                 512 * (j + 1)]
                    if USE_DR:
                        nc.tensor.matmul(
                            ps[:, 512 * hh + w0:512 * hh + w0 + n_w],
                            lhsT, rhs, start=True, stop=True, perf_mode=DR,
                            tile_position=(32 * h, 0))
                    else:
                        for pl in range(2):
                            nc.tensor.matmul(
                                ps[:, 512 * hh + w0:512 * hh + w0 + n_w],
                                lhsT[:, pl], rhs[:, pl],
                                start=(pl == 0), stop=(pl == 1),
                                tile_position=(32 * h, 0))

                def normalize(p, j, po, hh, direct=True):
                    # recip (psum -> sbuf, DVE) -> partition broadcast
                    # (gpsimd, sbuf-only: it cannot touch psum) -> DVE mul
                    sj = slice(512 * j, 512 * (j + 1))
                    rcp = NP.tile([1, 512], f32, tag="rc", bufs=2,
                                  name=f"rc_{p}_{j}_{hh}")
                    nc.vector.reciprocal(rcp[:], po[hh][64:65, :])
                    bs = NP.tile([64, 512], f32, tag="bs", bufs=2,
                                 name=f"bs_{p}_{j}_{hh}")
                    nc.gpsimd.partition_broadcast(bs[:], rcp[:])
                    nc.vector.tensor_tensor(
                        ao[p][64 * hh:64 * (hh + 1), sj],
                        po[hh][0:64, :], bs[:], MUL)

                # ---- cross-block pipelined attention stream ----
                # AVs flush in half-tile (single-hh) steps: consecutive
                # DoubleRow matmuls at different tile_positions crash the
                # PE, so every DR score pair is separated by a bf16 AV
                # matmul (or a tiny dummy while the pipeline fills)
                pend_q = []   # [et, i, w0, po, p, j, is_first, is_last, hh]

                def flush_half():
                    """Emit one AV matmul (one hh of the front pend tile)."""
                    item = pend_q[0]
                    et, i, w0, po, p, j, first, last, hh = item
                    h = 2 * p + hh
                    nc.tensor.matmul(
                        po[hh][:, w0:512],
                        vbig[:, 260 * i + 65 * h:260 * i + 65 * (h + 1)],
                        et[:, 512 * hh + w0:512 * hh + 512],
                        start=first, stop=last,
                        skip_group_check=True)
                    item[8] += 1
                    if item[8] == 2:
                        pend_q.pop(0)
                        if last:
                            normalize(p, j, po, 0, direct=True)
                            normalize(p, j, po, 1, direct=True)
                            if p == 1 and j < 3:
                                background.extend(
                                    (o_unit, (j, e)) for e in range(8))

                sep_ids = []

                def separator():
                    """bf16 PE matmul between two DR score matmuls."""
                    if len(pend_q) >= PEND_DEPTH:
                        flush_half()
                    else:
                        dpw = PSA.tile([128, 128], f32, tag="s", bufs=2,
                                       name=f"dmy_sep{len(sep_ids)}")
                        sep_ids.append(0)
                        nc.tensor.matmul(dpw[:], dmy[:, 0:128],
                                         dmy[:, 0:128], start=True, stop=True)

                # block order interleaves pass-1 (Act-rich, little PE work)
                # into the PE-heavy early pass-0 phase to balance engines;
                # (0,j) always precedes (1,j), and (1,3) stays last for the
                # final-drain/tail logic
                BLOCK_SEQ = ((0, 0), (0, 1), (0, 2), (1, 0), (0, 3),
                             (1, 1), (1, 2), (1, 3))
                for p, j in BLOCK_SEQ:
                    if True:
                        sj = slice(512 * j, 512 * (j + 1))
                        n_i = 4 * j + 4
                        po = [PSA.tile([65, 512], f32, tag=f"po{hh}",
                                       name=f"po_{hh}_{p}_{j}")
                              for hh in range(2)]
                        # ---- scores + exp, k-tile stream ----
                        for i in range(n_i):
                            di = i - 4 * j
                            cur_w0 = 0 if di < 0 else 128 * di
                            ps = PSA.tile([128, 1024], f32, tag="s",
                                          bufs=2, name=f"ps_s_{p}_{j}_{i}")
                            et = ET.tile([128, 1024], bf16, tag="et",
                                         bufs=ET_BUFS, name=f"et_{p}_{j}_{i}")
                            # pop BEFORE the AV flushes: bg writers (v st,
                            # rope) must be issued before their readers
                            if p == 0 or i % 2 == 1 or i < POP_FRONT:
                                pop_bg()
                                if p == 0 and j <= 1:
                                    pop_bg()
                            score_mm(p, 0, i, j, cur_w0, ps)
                            separator()
                            score_mm(p, 1, i, j, cur_w0, ps)
                            if di < 0:
                                nc.scalar.activation(et[:], ps[:], Exp)
                            else:
                                w0 = cur_w0
                                pssrc = ps[:].rearrange(
                                    "p (h w) -> p h w", h=2)[:, :, w0:512]
                                etdst = et[:].rearrange(
                                    "p (h w) -> p h w", h=2)[:, :, w0:512]
                                nc.scalar.activation(etdst, pssrc, Exp)
                                etwin = et[:].rearrange(
                                    "p (h w) -> p h w", h=2)[:, :, w0:w0 + 128]
                                triw = trit[:].rearrange(
                                    "p (h w) -> p h w", h=2)
                                nc.vector.tensor_tensor(etwin, etwin, triw,
                                                        MUL)
                            separator()
                            pend_q.append([et, i, cur_w0, po, p, j,
                                           i == 0, i == n_i - 1, 0])

                # ---- final drain: hh-split + clock-keeper dummies ----
                fin = list(pend_q)
                pend_q.clear()
                po_f, p_f, j_f = fin[0][3], fin[0][4], fin[0][5]
                for hh in range(2):
                    for et, i, w0, po, p, j, first, last_, done in fin:
                        if done > hh:
                            continue   # this half already flushed in-stream
                        nc.tensor.matmul(
                            po[hh][:, w0:512],
                            vbig[:, 260 * i + 65 * (2 * p + hh):
                                 260 * i + 65 * (2 * p + hh) + 65],
                            et[:, 512 * hh + w0:512 * hh + 512],
                            start=first, stop=last_,
                            skip_group_check=True)
                    normalize(p_f, j_f, po_f, hh, direct=True)
                for w in range(TAIL_DUMMIES):
                    dpw = PSA.tile([128, 128], f32, tag="s",
                                   bufs=2, name=f"dmy_tail{w}")
                    nc.tensor.matmul(dpw[:], dmy[:, 0:128], dmy[:, 0:128],
                                     start=True, stop=True)
                while background:
                    pop_bg()
                # tail: chunk-3 out-projection; psum tags rotate over every
                # free bank so the kc=0 halves all run early; e-chunks merge
                # into two 4-chunk staging tiles -> only 2 tail DMAs (HWDGE
                # descriptor-gen is 625ns each, serial)
                tail_tags = ("bg0", "bg1", "s", "s", "po0", "po1",
                             "bg0", "bg1")
                sj3 = slice(512 * 3, 512 * 4)
                for g in range(4):
                    queue = (nc.sync, nc.scalar)[g % 2]
                    ot2 = OS.tile([128, 1024], bf16, tag="ot4", bufs=4,
                                  name=f"ot2_{g}")
                    for k in range(2):
                        e = 2 * g + k
                        pf = PSA.tile([128, 512], f32, tag=tail_tags[e],
                                      bufs=2 if tail_tags[e] == "s" else 1,
                                      name=f"pf_3_{e}")
                        for kc in range(2):
                            nc.tensor.matmul(
                                pf[:], wo_sb[kc][:, 128 * e:128 * (e + 1)],
                                ao[kc][:, sj3],
                                start=(kc == 0), stop=(kc == 1))
                        if k % 2 == 0:
                            nc.vector.tensor_copy(
                                ot2[:, 512 * k:512 * (k + 1)], pf[:])
                        else:
                            nc.scalar.copy(
                                ot2[:, 512 * k:512 * (k + 1)], pf[:])
                    dst = oT[256 * g:256 * (g + 1), sj3].rearrange(
                        "(e p) c -> p e c", e=2)
                    src = ot2[:].rearrange("p (e c) -> p e c", e=2)
                    queue.dma_start(out=dst, in_=src)

    nc.finalize()
    return nc


# --------------------------------------------------------------------------
# Host-side input prep / output assembly
# --------------------------------------------------------------------------

def prep_core_inputs(x, qkv_w, out_w, token_positions, S=2048):
    """Build the 8 per-core input maps (numpy, host-side sharding)."""
    import ml_dtypes
    bf16 = ml_dtypes.bfloat16

    x = np.asarray(x, dtype=np.float32)
    qkv_w = np.asarray(qkv_w, dtype=np.float32)
    out_w = np.asarray(out_w, dtype=np.float32)
    pos = np.asarray(token_positions).astype(np.float32)

    B = x.shape[0]
    inv_freq = 1.0 / (ROPE_THETA ** (np.arange(0, DK, 2, dtype=np.float32) / DK))
    # plane tables: row p (p%32 = m), plane i -> freq[(32i + m)//2]
    # layout [sin0 S | cos0 S | sin1 S | cos1 S], rows tiled x4 heads
    cs2 = np.empty((128, 4 * S), dtype=np.float32)
    # signed sin for the pair-swap shuffle rope: q' = q*cos + swap(q)*sin'
    # with sin'[2a] = -sin_a (even rows), sin'[2a+1] = +sin_a
    signs = np.where(np.arange(32) % 2 == 0, -1.0, 1.0).astype(np.float32)
    for i in range(2):
        freqs = inv_freq[np.repeat(np.arange(16), 2) + 16 * i]   # [32] dup'd
        ang = pos[:, None] * freqs[None, :]                      # [S, 32]
        sin_b = np.tile(signs[:, None] * np.sin(ang).T, (4, 1))  # [128, S]
        cos_b = np.tile(np.cos(ang).T, (4, 1))
        cs2[:, (2 * i) * S:(2 * i + 1) * S] = sin_b
        cs2[:, (2 * i + 1) * S:(2 * i + 2) * S] = cos_b
    cs2 = np.ascontiguousarray(cs2).astype(bf16)

    tri1 = (np.arange(128)[None, :] >= np.arange(128)[:, None]).astype(np.float32)
    tri = np.ascontiguousarray(np.concatenate([tri1, tri1], axis=1)).astype(bf16)

    xT = [np.ascontiguousarray(x[b].T).astype(bf16) for b in range(B)]  # [D, S]

    scale = 1.0 / np.sqrt(np.float32(DK))

    def sbuf_interleave(w):
        """[256 out-dims, 1024] -> transposed tiles [128, t*256]."""
        wT = np.ascontiguousarray(w.T)                        # [1024, 256]
        return np.ascontiguousarray(np.concatenate(
            [wT[128 * t:128 * (t + 1)] for t in range(8)], axis=1)).astype(bf16)

    # plane-i permutation: stationary col c = 32h + m -> local row 64h + 32i + m
    perm = [np.array([64 * h + 32 * i + m for h in range(4) for m in range(32)])
            for i in range(2)]

    in_maps = []
    for c in range(N_CORES):
        b = c // 4
        g = c % 4
        hsl = slice(64 * H_LOC * g, 64 * H_LOC * (g + 1))     # 256 dims
        wq = qkv_w[0 * D:1 * D][hsl] * scale                  # [256, 1024]
        wk = qkv_w[1 * D:2 * D][hsl]
        wv = qkv_w[2 * D:3 * D][hsl]
        wqp = [np.concatenate([wq[perm[i]], wk[perm[i]]], axis=0)
               for i in range(2)]                              # [256, 1024]
        in_maps.append({
            "xT": xT[b],
            "wq0T": sbuf_interleave(wqp[0]),
            "wq1T": sbuf_interleave(wqp[1]),
            "wvT": sbuf_interleave(wv),
            "woT": np.ascontiguousarray(out_w[:, hsl].T).astype(bf16),
            "cs2T": cs2,
            "tri": tri,
        })
    return in_maps


def assemble_output(results, B=2, S=2048):
    """Sum per-core partial oT [D, S] over each batch's 4 cores, transpose."""
    out = np.empty((B, S, D), dtype=np.float32)
    for b in range(B):
        acc = results[4 * b]["oT"].astype(np.float32)
        for g in range(1, 4):
            acc = acc + results[4 * b + g]["oT"].astype(np.float32)
        out[b] = acc.T
    return out


_NC_CACHE = {}


def get_nc(S=2048):
    if S not in _NC_CACHE:
        _NC_CACHE[S] = build_nc(S)
    return _NC_CACHE[S]


def kernel(x, qkv_w, out_w, token_positions):
    _ensure_repo_on_path()
    from concourse.bass_utils import run_bass_kernel_spmd

    x = np.asarray(x)
    S = x.shape[1]
    in_maps = prep_core_inputs(x, qkv_w, out_w, token_positions, S=S)
    nc = get_nc(S)
    res = run_bass_kernel_spmd(nc, in_maps, core_ids=list(range(N_CORES)))
    return assemble_output(res.results, B=x.shape[0], S=S)
